# revision 7
# baseline (speedup 1.0000x reference)
"""Trainium2 Bass/Tile kernel for nn_PolarizingBlock (masked-mean polarizing block).

Computation (per batch b):
    Ar/Ai  = masked mean of Z_real/Z_imag over T        (B,1,D)
    mag    = sqrt(Ar^2+Ai^2); log_mag = log(mag+eps)
    psi_mag MLP (1->H->1, gelu) on log_mag -> mag_delta
    psi_phase MLP (2->H->2, gelu) on normalized (Ar,Ai) -> unit vec pv
    r_out  = exp(log_mag + mag_scale*mag_delta)
    out    = stack([Z_real + r_out*pv0, Z_imag + r_out*pv1])  (2,B,T,D)

Sharding: data-parallel over batch. 8 cores x 2 batches each. Each core
streams its 2 batches of Z twice (once for the masked sum via PE matmuls
with the mask column as stationary weights, once for the broadcast add)
-> ~192MB of HBM traffic per core, memory-bound.
"""

import numpy as np

import concourse.bass as bass
import concourse.bacc as bacc
import concourse.tile as tile
from concourse import mybir
from concourse.bass_utils import run_bass_kernel_spmd
from concourse.masks import make_identity

B, T, D, H = 16, 4096, 1024, 32
NCORES = 8
NB = B // NCORES  # batches per core
EPS = 1e-6
F32 = mybir.dt.float32
AL = mybir.AluOpType
AF = mybir.ActivationFunctionType

TCHUNK = 512            # T rows per streamed tile (2MB per tile)
KSUB = TCHUNK // 128    # 4 column-groups of 128 T-rows each
NCH = T // TCHUNK       # 8 chunks per batch
HD = D // 2             # 512 = max matmul moving free dim


def build_kernel(sim_gelu=False):
    """sim_gelu=True replaces the Gelu LUT (not implemented in CoreSim) with
    the tanh approximation built from primitives - for simulator checks only."""
    nc = bacc.Bacc("TRN2", target_bir_lowering=False, debug=False)

    zr = nc.dram_tensor("zr", (NB, T, D), F32, kind="ExternalInput").ap()
    zi = nc.dram_tensor("zi", (NB, T, D), F32, kind="ExternalInput").ap()
    mask = nc.dram_tensor("mask", (NB, T), F32, kind="ExternalInput").ap()
    w1m = nc.dram_tensor("w1m", (H, 1), F32, kind="ExternalInput").ap()
    b1m = nc.dram_tensor("b1m", (H, 1), F32, kind="ExternalInput").ap()
    w2m = nc.dram_tensor("w2m", (H, 1), F32, kind="ExternalInput").ap()
    w1p0 = nc.dram_tensor("w1p0", (H, 1), F32, kind="ExternalInput").ap()
    w1p1 = nc.dram_tensor("w1p1", (H, 1), F32, kind="ExternalInput").ap()
    b1p = nc.dram_tensor("b1p", (H, 1), F32, kind="ExternalInput").ap()
    w2p0 = nc.dram_tensor("w2p0", (H, 1), F32, kind="ExternalInput").ap()
    w2p1 = nc.dram_tensor("w2p1", (H, 1), F32, kind="ExternalInput").ap()
    b2m = nc.dram_tensor("b2m", (1, 1), F32, kind="ExternalInput").ap()
    b2p0 = nc.dram_tensor("b2p0", (1, 1), F32, kind="ExternalInput").ap()
    b2p1 = nc.dram_tensor("b2p1", (1, 1), F32, kind="ExternalInput").ap()
    msc = nc.dram_tensor("msc", (1, 1), F32, kind="ExternalInput").ap()
    out = nc.dram_tensor("out", (2, NB, T, D), F32, kind="ExternalOutput").ap()

    with tile.TileContext(nc) as tc:
        with (
            tc.tile_pool(name="consts", bufs=1) as consts,
            tc.tile_pool(name="za", bufs=4) as za,
            tc.tile_pool(name="zc", bufs=3) as zc,
            tc.tile_pool(name="rows", bufs=10) as rows,
            tc.tile_pool(name="hpool", bufs=1) as hpool,
            tc.tile_pool(name="arep", bufs=1) as arepp,
            tc.tile_pool(name="pacc", bufs=1, space="PSUM") as pacc,
            tc.tile_pool(name="pmisc", bufs=3, space="PSUM") as pmisc,
        ):
            # ---- constants ----
            id32 = consts.tile([32, 32], F32, name="id32", tag="id32")
            make_identity(nc, id32)
            ones32 = consts.tile([1, 32], F32, name="ones32", tag="ones32")
            nc.vector.memset(ones32, 1.0)
            ones128 = consts.tile([1, 128], F32, name="ones128", tag="ones128")
            nc.vector.memset(ones128, 1.0)
            onecol = consts.tile([128, 1], F32, name="onecol", tag="onecol")
            nc.vector.memset(onecol, 1.0)
            eps1 = consts.tile([1, 1], F32, name="eps1", tag="eps1")
            nc.vector.memset(eps1, EPS)

            w1m_s = consts.tile_from(w1m, name="w1m_s")
            b1m_s = consts.tile_from(b1m, name="b1m_s")
            w2m_s = consts.tile_from(w2m, name="w2m_s")
            w1p0_s = consts.tile_from(w1p0, name="w1p0_s")
            w1p1_s = consts.tile_from(w1p1, name="w1p1_s")
            b1p_s = consts.tile_from(b1p, name="b1p_s")
            w2p0_s = consts.tile_from(w2p0, name="w2p0_s")
            w2p1_s = consts.tile_from(w2p1, name="w2p1_s")
            b2m_s = consts.tile_from(b2m, name="b2m_s")
            b2p0_s = consts.tile_from(b2p0, name="b2p0_s")
            b2p1_s = consts.tile_from(b2p1, name="b2p1_s")
            msc_s = consts.tile_from(msc, name="msc_s")

            # ---- mask columns: (T,) -> (128, 32) where col c = T-tile c ----
            # and count = clip(sum(mask), 1) -> inv = 1/count
            mask_cols, invs = [], []
            for b in range(NB):
                mask_rm = consts.tile(
                    [32, 128], F32, name=f"mask_rm{b}", tag=f"mask_rm{b}"
                )
                nc.sync.dma_start(
                    out=mask_rm, in_=mask[b].rearrange("(r q) -> r q", q=128)
                )
                mask_ps = pmisc.tile([128, 32], F32, name=f"mask_ps{b}", tag="m")
                nc.tensor.transpose(mask_ps, mask_rm, id32)
                mc = consts.tile([128, 32], F32, name=f"mask_c{b}", tag=f"mask_c{b}")
                nc.vector.tensor_copy(mc, mask_ps)
                mask_cols.append(mc)

                cnt_pp = consts.tile([128, 1], F32, name=f"cntpp{b}", tag=f"cntpp{b}")
                nc.vector.reduce_sum(cnt_pp, mc, axis=mybir.AxisListType.X)
                cnt_ps = pmisc.tile([1, 1], F32, name=f"cntps{b}", tag="m")
                nc.tensor.matmul(cnt_ps, cnt_pp, onecol)
                inv_b = consts.tile([1, 1], F32, name=f"inv{b}", tag=f"inv{b}")
                nc.vector.tensor_scalar(inv_b, cnt_ps, 1.0, None, AL.max)
                nc.vector.reciprocal(inv_b, inv_b)
                invs.append(inv_b)

            def gelu(xt, b, nm):
                """In-place gelu on (H, D) tile xt."""
                if not sim_gelu:
                    nc.scalar.activation(xt, xt, AF.Gelu)
                    return
                # 0.5*x*(1+tanh(sqrt(2/pi)*(x+0.044715*x^3)))
                x2 = hpool.tile([H, D], F32, name=f"g2{nm}{b}", tag="gtmp", bufs=2)
                nc.scalar.activation(x2, xt, AF.Square)
                nc.vector.tensor_mul(x2, x2, xt)  # x^3
                nc.vector.scalar_tensor_tensor(
                    out=x2, in0=x2, scalar=0.044715, in1=xt,
                    op0=AL.mult, op1=AL.add,
                )
                nc.scalar.activation(
                    x2, x2, AF.Tanh, scale=float(np.sqrt(2.0 / np.pi))
                )
                nc.vector.tensor_scalar(x2, x2, 0.5, 0.5, AL.mult, AL.add)
                nc.vector.tensor_mul(xt, xt, x2)

            def phase_a(b):
                """Masked sums over T via PE; returns psum (1, D) tiles."""
                ar_ps = pacc.tile([1, D], F32, name=f"ar_ps{b}", tag="ar")
                ai_ps = pacc.tile([1, D], F32, name=f"ai_ps{b}", tag="ai")
                for i in range(NCH):
                    sl = slice(i * TCHUNK, (i + 1) * TCHUNK)
                    ztr = za.tile([128, KSUB, D], F32, name=f"ztr_{b}_{i}", tag="za")
                    nc.sync.dma_start(
                        out=ztr, in_=zr[b][sl].rearrange("(k p) d -> p k d", p=128)
                    )
                    zti = za.tile([128, KSUB, D], F32, name=f"zti_{b}_{i}", tag="za")
                    nc.sync.dma_start(
                        out=zti, in_=zi[b][sl].rearrange("(k p) d -> p k d", p=128)
                    )
                    for k in range(KSUB):
                        c = i * KSUB + k
                        lhs = mask_cols[b][:, c : c + 1]
                        st = (i == 0) and (k == 0)
                        sp = (i == NCH - 1) and (k == KSUB - 1)
                        for ps, zt in ((ar_ps, ztr), (ai_ps, zti)):
                            nc.tensor.matmul(
                                ps[:, 0:HD], lhs, zt[:, k, 0:HD], start=st, stop=sp
                            )
                            nc.tensor.matmul(
                                ps[:, HD:D], lhs, zt[:, k, HD:D], start=st, stop=sp
                            )
                return ar_ps, ai_ps

            def row(name):
                return rows.tile([1, D], F32, name=name, tag="row")

            def phase_b(b, ar_ps, ai_ps):
                """Tiny MLPs on the means; returns (anr, ani) (1, D) sbuf rows."""
                inv_b = invs[b]
                ar = row(f"ar{b}")
                nc.vector.tensor_scalar(ar, ar_ps, inv_b, None, AL.mult)
                ai = row(f"ai{b}")
                nc.vector.tensor_scalar(ai, ai_ps, inv_b, None, AL.mult)

                m2 = row(f"m2{b}")
                nc.scalar.activation(m2, ar, AF.Square)
                t0 = row(f"t0{b}")
                nc.scalar.activation(t0, ai, AF.Square)
                nc.vector.tensor_add(m2, m2, t0)
                mag = row(f"mag{b}")
                nc.scalar.activation(mag, m2, AF.Sqrt)
                logm = row(f"logm{b}")
                nc.scalar.activation(logm, mag, AF.Ln, bias=eps1)
                nc.vector.tensor_scalar_add(mag, mag, EPS)
                minv = row(f"minv{b}")
                nc.vector.reciprocal(minv, mag)
                phr = row(f"phr{b}")
                nc.vector.tensor_mul(phr, ar, minv)
                phi = row(f"phi{b}")
                nc.vector.tensor_mul(phi, ai, minv)

                # hidden layers: H on partitions via ones-weights replication
                hm = hpool.tile([H, D], F32, name=f"hm{b}", tag=f"hm{b}")
                hp = hpool.tile([H, D], F32, name=f"hp{b}", tag=f"hp{b}")
                for hf in range(2):
                    cs = slice(hf * HD, (hf + 1) * HD)
                    rep = pmisc.tile([H, HD], F32, name=f"repm{b}{hf}", tag="m")
                    nc.tensor.matmul(rep, ones32, logm[:, cs])
                    nc.vector.tensor_scalar(
                        hm[:, cs], rep, w1m_s, b1m_s, AL.mult, AL.add
                    )
                    repr_ = pmisc.tile([H, HD], F32, name=f"repr{b}{hf}", tag="m")
                    nc.tensor.matmul(repr_, ones32, phr[:, cs])
                    nc.vector.tensor_scalar(
                        hp[:, cs], repr_, w1p0_s, b1p_s, AL.mult, AL.add
                    )
                    repi = pmisc.tile([H, HD], F32, name=f"repi{b}{hf}", tag="m")
                    nc.tensor.matmul(repi, ones32, phi[:, cs])
                    nc.vector.scalar_tensor_tensor(
                        out=hp[:, cs],
                        in0=repi,
                        scalar=w1p1_s,
                        in1=hp[:, cs],
                        op0=AL.mult,
                        op1=AL.add,
                    )
                gelu(hm, b, "m")
                gelu(hp, b, "p")

                # H -> scalar reductions via PE with W2 columns as weights
                magd = row(f"magd{b}")
                pv0 = row(f"pv0{b}")
                pv1 = row(f"pv1{b}")
                for dst, wcol, bias, src in (
                    (magd, w2m_s, b2m_s, hm),
                    (pv0, w2p0_s, b2p0_s, hp),
                    (pv1, w2p1_s, b2p1_s, hp),
                ):
                    for hf in range(2):
                        cs = slice(hf * HD, (hf + 1) * HD)
                        red = pmisc.tile([1, HD], F32, name=f"red{b}{hf}", tag="m")
                        nc.tensor.matmul(red, wcol, src[:, cs])
                        nc.vector.tensor_scalar(dst[:, cs], red, bias, None, AL.add)

                # normalize pv, r_out, A_new
                q0 = row(f"q0{b}")
                nc.scalar.activation(q0, pv0, AF.Square)
                q1 = row(f"q1{b}")
                nc.scalar.activation(q1, pv1, AF.Square)
                nc.vector.tensor_add(q0, q0, q1)
                nc.scalar.activation(q1, q0, AF.Sqrt)
                nc.vector.tensor_scalar(q1, q1, 1e-12, None, AL.max)
                nc.vector.reciprocal(q0, q1)  # q0 = 1/max(norm, 1e-12)
                nc.vector.tensor_mul(pv0, pv0, q0)
                nc.vector.tensor_mul(pv1, pv1, q0)

                nc.vector.scalar_tensor_tensor(
                    out=magd, in0=magd, scalar=msc_s, in1=logm,
                    op0=AL.mult, op1=AL.add,
                )  # magd = log_mag + msc*mag_delta
                r_out = row(f"r{b}")
                nc.scalar.activation(r_out, magd, AF.Exp)
                anr = row(f"anr{b}")
                nc.vector.tensor_mul(anr, r_out, pv0)
                ani = row(f"ani{b}")
                nc.vector.tensor_mul(ani, r_out, pv1)
                return anr, ani

            def replicate(b, ri, src):
                """(1, D) row -> (128, D) sbuf tile via ones-weights matmul."""
                at = arepp.tile(
                    [128, D], F32, name=f"arep{ri}{b}", tag=f"arep{ri}{b}"
                )
                for hf in range(2):
                    cs = slice(hf * HD, (hf + 1) * HD)
                    rs = pmisc.tile([128, HD], F32, name=f"rs{ri}{b}{hf}", tag="m")
                    nc.tensor.matmul(rs, ones128, src[:, cs])
                    nc.vector.tensor_copy(at[:, cs], rs)
                return at

            def phase_c(b, ri, zsrc, at):
                for i in range(NCH):
                    sl = slice(i * TCHUNK, (i + 1) * TCHUNK)
                    zt = zc.tile([128, KSUB, D], F32, name=f"zo{ri}{b}{i}", tag="zc")
                    nc.sync.dma_start(
                        out=zt, in_=zsrc[b][sl].rearrange("(k p) d -> p k d", p=128)
                    )
                    for k in range(KSUB):
                        nc.vector.tensor_add(zt[:, k, :], zt[:, k, :], at)
                    nc.sync.dma_start(
                        out=out[ri, b][sl].rearrange("(k p) d -> p k d", p=128),
                        in_=zt,
                    )

            # ---- schedule: A0 B0 C0 | A1 B1 C1 (C0 DMA overlaps B1/A1) ----
            for b in range(NB):
                ar_ps, ai_ps = phase_a(b)
                anr, ani = phase_b(b, ar_ps, ai_ps)
                arr = replicate(b, 0, anr)
                ari = replicate(b, 1, ani)
                phase_c(b, 0, zr, arr)
                phase_c(b, 1, zi, ari)

    nc.compile()
    return nc


_NC = None


def _get_nc():
    global _NC
    if _NC is None:
        _NC = build_kernel()
    return _NC


def make_in_maps(Z_real, Z_imag, mask, W1m, b1m, W2m, b2m, W1p, b1p, W2p, b2p,
                 mag_scale):
    f = np.float32
    consts = {
        "w1m": np.ascontiguousarray(np.asarray(W1m, f).reshape(H, 1)),
        "b1m": np.ascontiguousarray(np.asarray(b1m, f).reshape(H, 1)),
        "w2m": np.ascontiguousarray(np.asarray(W2m, f).reshape(H, 1)),
        "w1p0": np.ascontiguousarray(np.asarray(W1p, f)[:, 0].reshape(H, 1)),
        "w1p1": np.ascontiguousarray(np.asarray(W1p, f)[:, 1].reshape(H, 1)),
        "b1p": np.ascontiguousarray(np.asarray(b1p, f).reshape(H, 1)),
        "w2p0": np.ascontiguousarray(np.asarray(W2p, f)[0].reshape(H, 1)),
        "w2p1": np.ascontiguousarray(np.asarray(W2p, f)[1].reshape(H, 1)),
        "b2m": np.asarray(b2m, f).reshape(1, 1).copy(),
        "b2p0": np.asarray(b2p, f).reshape(-1)[0].reshape(1, 1).copy(),
        "b2p1": np.asarray(b2p, f).reshape(-1)[1].reshape(1, 1).copy(),
        "msc": np.asarray(mag_scale, f).reshape(1, 1).copy(),
    }
    Z_real = np.asarray(Z_real, f)
    Z_imag = np.asarray(Z_imag, f)
    mask = np.asarray(mask, f)
    in_maps = []
    for c in range(NCORES):
        bs = slice(c * NB, (c + 1) * NB)
        in_maps.append(
            {
                "zr": np.ascontiguousarray(Z_real[bs]),
                "zi": np.ascontiguousarray(Z_imag[bs]),
                "mask": np.ascontiguousarray(mask[bs]),
                **consts,
            }
        )
    return in_maps


def kernel(**inputs):
    nc = _get_nc()
    in_maps = make_in_maps(**inputs)
    res = run_bass_kernel_spmd(nc, in_maps, core_ids=list(range(NCORES)))
    out = np.empty((2, B, T, D), np.float32)
    for c in range(NCORES):
        out[:, c * NB : (c + 1) * NB] = res.results[c]["out"]
    return out


# revision 21
# speedup vs baseline: 3.1600x; 3.1600x over previous
"""Trainium2 Bass/Tile kernel for nn_PolarizingBlock (masked-mean polarizing block).

Computation (per batch b):
    Ar/Ai  = masked mean of Z_real/Z_imag over T        (B,1,D)
    mag    = sqrt(Ar^2+Ai^2); log_mag = log(mag+eps)
    psi_mag MLP (1->H->1, gelu) on log_mag -> mag_delta
    psi_phase MLP (2->H->2, gelu) on normalized (Ar,Ai) -> unit vec pv
    r_out  = exp(log_mag + mag_scale*mag_delta)
    out    = stack([Z_real + r_out*pv0, Z_imag + r_out*pv1])  (2,B,T,D)

Sharding: data-parallel over batch. 8 cores x 2 batches each. Each core
streams its 2 batches of Z twice (once for the masked sum via PE matmuls
with the mask column as stationary weights, once for the broadcast add)
-> ~192MB of HBM traffic per core, memory-bound.
"""

import numpy as np

import concourse.bass as bass
import concourse.bacc as bacc
import concourse.tile as tile
from concourse import mybir
from concourse.bass_utils import run_bass_kernel_spmd
from concourse.masks import make_identity

B, T, D, H = 16, 4096, 1024, 32
NCORES = 8
NB = B // NCORES  # batches per core
EPS = 1e-6
F32 = mybir.dt.float32
F32R = mybir.dt.float32r
AL = mybir.AluOpType
AF = mybir.ActivationFunctionType

TCHUNK = 512            # T rows per streamed tile (2MB per tile)
KSUB = TCHUNK // 128    # 4 column-groups of 128 T-rows each
NCH = T // TCHUNK       # 8 chunks per batch
HD = D // 2             # 512 = max matmul moving free dim
KEEP_CHUNKS = 3         # trailing chunks of the last batch kept in SBUF for phase C


def build_kernel(sim_gelu=False, reps=1):
    """sim_gelu=True replaces the Gelu LUT (not implemented in CoreSim) with
    the tanh approximation built from primitives - for simulator checks only.
    reps>1 emits the whole computation multiple times in one NEFF (device-side
    timing: the T(reps)-T(1) slope cancels dispatch overhead)."""
    nc = bacc.Bacc("TRN2", target_bir_lowering=False, debug=False)

    zr = nc.dram_tensor("zr", (NB, T, D), F32, kind="ExternalInput").ap()
    zi = nc.dram_tensor("zi", (NB, T, D), F32, kind="ExternalInput").ap()
    mask = nc.dram_tensor("mask", (NB, T), F32, kind="ExternalInput").ap()
    w1m = nc.dram_tensor("w1m", (H, 1), F32, kind="ExternalInput").ap()
    b1m = nc.dram_tensor("b1m", (H, 1), F32, kind="ExternalInput").ap()
    w2m = nc.dram_tensor("w2m", (H, 1), F32, kind="ExternalInput").ap()
    w1p0 = nc.dram_tensor("w1p0", (H, 1), F32, kind="ExternalInput").ap()
    w1p1 = nc.dram_tensor("w1p1", (H, 1), F32, kind="ExternalInput").ap()
    b1p = nc.dram_tensor("b1p", (H, 1), F32, kind="ExternalInput").ap()
    w2p0 = nc.dram_tensor("w2p0", (H, 1), F32, kind="ExternalInput").ap()
    w2p1 = nc.dram_tensor("w2p1", (H, 1), F32, kind="ExternalInput").ap()
    b2m = nc.dram_tensor("b2m", (1, 1), F32, kind="ExternalInput").ap()
    b2p0 = nc.dram_tensor("b2p0", (1, 1), F32, kind="ExternalInput").ap()
    b2p1 = nc.dram_tensor("b2p1", (1, 1), F32, kind="ExternalInput").ap()
    msc = nc.dram_tensor("msc", (1, 1), F32, kind="ExternalInput").ap()
    out = nc.dram_tensor("out", (2, NB, T, D), F32, kind="ExternalOutput").ap()

    with tile.TileContext(nc) as tc:
        with (
            tc.tile_pool(name="consts", bufs=1) as consts,
            tc.tile_pool(name="zpool", bufs=7) as zpool,
            tc.tile_pool(name="rows", bufs=10) as rows,
            tc.tile_pool(name="hpool", bufs=1) as hpool,
            tc.tile_pool(name="arep", bufs=1) as arepp,
            tc.tile_pool(name="pacc", bufs=1, space="PSUM") as pacc,
            tc.tile_pool(name="pmisc", bufs=3, space="PSUM") as pmisc,
        ):
            # ---- constants ----
            id32 = consts.tile([32, 32], F32, name="id32", tag="id32")
            make_identity(nc, id32)
            ones32 = consts.tile([1, 32], F32, name="ones32", tag="ones32")
            nc.vector.memset(ones32, 1.0)
            ones128 = consts.tile([1, 128], F32, name="ones128", tag="ones128")
            nc.vector.memset(ones128, 1.0)
            onecol = consts.tile([128, 1], F32, name="onecol", tag="onecol")
            nc.vector.memset(onecol, 1.0)
            eps1 = consts.tile([1, 1], F32, name="eps1", tag="eps1")
            nc.vector.memset(eps1, EPS)

            w1m_s = consts.tile_from(w1m, name="w1m_s")
            b1m_s = consts.tile_from(b1m, name="b1m_s")
            w2m_s = consts.tile_from(w2m, name="w2m_s")
            w1p0_s = consts.tile_from(w1p0, name="w1p0_s")
            w1p1_s = consts.tile_from(w1p1, name="w1p1_s")
            b1p_s = consts.tile_from(b1p, name="b1p_s")
            w2p0_s = consts.tile_from(w2p0, name="w2p0_s")
            w2p1_s = consts.tile_from(w2p1, name="w2p1_s")
            b2m_s = consts.tile_from(b2m, name="b2m_s")
            b2p0_s = consts.tile_from(b2p0, name="b2p0_s")
            b2p1_s = consts.tile_from(b2p1, name="b2p1_s")
            msc_s = consts.tile_from(msc, name="msc_s")

            # ---- mask columns: (T,) -> (128, 32) where col c = T-tile c ----
            # and count = clip(sum(mask), 1) -> inv = 1/count
            mask_cols, invs = [], []
            for b in range(NB):
                mask_rm = consts.tile(
                    [32, 128], F32, name=f"mask_rm{b}", tag=f"mask_rm{b}"
                )
                nc.sync.dma_start(
                    out=mask_rm, in_=mask[b].rearrange("(r q) -> r q", q=128)
                )
                mask_ps = pmisc.tile([128, 32], F32, name=f"mask_ps{b}", tag="m")
                nc.tensor.transpose(mask_ps, mask_rm, id32)
                mc = consts.tile([128, 32], F32, name=f"mask_c{b}", tag=f"mask_c{b}")
                nc.vector.tensor_copy(mc, mask_ps)
                # f32r copy of the mask columns for the phase-A matmul weights
                mcr = consts.tile(
                    [128, 32], F32R, name=f"mask_cr{b}", tag=f"mask_cr{b}"
                )
                nc.vector.tensor_copy(mcr, mask_ps)
                mask_cols.append(mcr)

                cnt_pp = consts.tile([128, 1], F32, name=f"cntpp{b}", tag=f"cntpp{b}")
                nc.vector.reduce_sum(cnt_pp, mc, axis=mybir.AxisListType.X)
                cnt_ps = pmisc.tile([1, 1], F32, name=f"cntps{b}", tag="m")
                nc.tensor.matmul(cnt_ps, cnt_pp, onecol)
                inv_b = consts.tile([1, 1], F32, name=f"inv{b}", tag=f"inv{b}")
                nc.vector.tensor_scalar(inv_b, cnt_ps, 1.0, None, AL.max)
                nc.vector.reciprocal(inv_b, inv_b)
                invs.append(inv_b)

            def gelu(xt, b, nm):
                """In-place gelu on (H, D) tile xt."""
                if not sim_gelu:
                    nc.scalar.activation(xt, xt, AF.Gelu)
                    return
                # 0.5*x*(1+tanh(sqrt(2/pi)*(x+0.044715*x^3)))
                x2 = hpool.tile([H, D], F32, name=f"g2{nm}{b}", tag="gtmp", bufs=2)
                nc.scalar.activation(x2, xt, AF.Square)
                nc.vector.tensor_mul(x2, x2, xt)  # x^3
                nc.vector.scalar_tensor_tensor(
                    out=x2, in0=x2, scalar=0.044715, in1=xt,
                    op0=AL.mult, op1=AL.add,
                )
                nc.scalar.activation(
                    x2, x2, AF.Tanh, scale=float(np.sqrt(2.0 / np.pi))
                )
                nc.vector.tensor_scalar(x2, x2, 0.5, 0.5, AL.mult, AL.add)
                nc.vector.tensor_mul(xt, xt, x2)

            def phase_a(b, keep=None):
                """Masked sums over T via PE; returns psum (1, D) tiles.
                keep: dict to retain the last chunks' SBUF tiles for reuse
                in phase C (skips their re-load from HBM)."""
                ar_ps = pacc.tile([1, D], F32, name=f"ar_ps{b}", tag="ar")
                ai_ps = pacc.tile([1, D], F32, name=f"ai_ps{b}", tag="ai")
                for i in range(NCH):
                    sl = slice(i * TCHUNK, (i + 1) * TCHUNK)
                    # f32r-typed tiles (bitwise load): fp32r matmul runs the
                    # PE at full rate vs 1/4 for fp32; sums of ~N(0,1) stay
                    # accurate via the fp32 PSUM accumulate
                    ztr = zpool.tile([128, KSUB, D], F32R, name=f"ztr_{b}_{i}", tag="z")
                    nc.sync.dma_start(
                        out=ztr,
                        in_=zr[b][sl].rearrange("(k p) d -> p k d", p=128).bitcast(F32R),
                    )
                    zti = zpool.tile([128, KSUB, D], F32R, name=f"zti_{b}_{i}", tag="z")
                    nc.sync.dma_start(
                        out=zti,
                        in_=zi[b][sl].rearrange("(k p) d -> p k d", p=128).bitcast(F32R),
                    )
                    for k in range(KSUB):
                        c = i * KSUB + k
                        lhs = mask_cols[b][:, c : c + 1]
                        st = (i == 0) and (k == 0)
                        sp = (i == NCH - 1) and (k == KSUB - 1)
                        for ps, zt in ((ar_ps, ztr), (ai_ps, zti)):
                            nc.tensor.matmul(
                                ps[:, 0:HD], lhs, zt[:, k, 0:HD], start=st, stop=sp
                            )
                            nc.tensor.matmul(
                                ps[:, HD:D], lhs, zt[:, k, HD:D], start=st, stop=sp
                            )
                    if keep is not None and i >= NCH - KEEP_CHUNKS:
                        keep[(0, i)] = ztr
                        keep[(1, i)] = zti
                return ar_ps, ai_ps

            def row(name):
                return rows.tile([1, D], F32, name=name, tag="row")

            def phase_b(b, ar_ps, ai_ps):
                """Tiny MLPs on the means; returns (anr, ani) (1, D) sbuf rows."""
                inv_b = invs[b]
                ar = row(f"ar{b}")
                nc.vector.tensor_scalar(ar, ar_ps, inv_b, None, AL.mult)
                ai = row(f"ai{b}")
                nc.vector.tensor_scalar(ai, ai_ps, inv_b, None, AL.mult)

                m2 = row(f"m2{b}")
                nc.scalar.activation(m2, ar, AF.Square)
                t0 = row(f"t0{b}")
                nc.scalar.activation(t0, ai, AF.Square)
                nc.vector.tensor_add(m2, m2, t0)
                mag = row(f"mag{b}")
                nc.scalar.activation(mag, m2, AF.Sqrt)
                logm = row(f"logm{b}")
                nc.scalar.activation(logm, mag, AF.Ln, bias=eps1)
                nc.vector.tensor_scalar_add(mag, mag, EPS)
                minv = row(f"minv{b}")
                nc.vector.reciprocal(minv, mag)
                phr = row(f"phr{b}")
                nc.vector.tensor_mul(phr, ar, minv)
                phi = row(f"phi{b}")
                nc.vector.tensor_mul(phi, ai, minv)

                # hidden layers: H on partitions via ones-weights replication
                hm = hpool.tile([H, D], F32, name=f"hm{b}", tag=f"hm{b}")
                hp = hpool.tile([H, D], F32, name=f"hp{b}", tag=f"hp{b}")
                for hf in range(2):
                    cs = slice(hf * HD, (hf + 1) * HD)
                    rep = pmisc.tile([H, HD], F32, name=f"repm{b}{hf}", tag="m")
                    nc.tensor.matmul(rep, ones32, logm[:, cs])
                    nc.vector.tensor_scalar(
                        hm[:, cs], rep, w1m_s, b1m_s, AL.mult, AL.add
                    )
                    repr_ = pmisc.tile([H, HD], F32, name=f"repr{b}{hf}", tag="m")
                    nc.tensor.matmul(repr_, ones32, phr[:, cs])
                    nc.vector.tensor_scalar(
                        hp[:, cs], repr_, w1p0_s, b1p_s, AL.mult, AL.add
                    )
                    repi = pmisc.tile([H, HD], F32, name=f"repi{b}{hf}", tag="m")
                    nc.tensor.matmul(repi, ones32, phi[:, cs])
                    nc.vector.scalar_tensor_tensor(
                        out=hp[:, cs],
                        in0=repi,
                        scalar=w1p1_s,
                        in1=hp[:, cs],
                        op0=AL.mult,
                        op1=AL.add,
                    )
                gelu(hm, b, "m")
                gelu(hp, b, "p")

                # H -> scalar reductions via PE with W2 columns as weights
                magd = row(f"magd{b}")
                pv0 = row(f"pv0{b}")
                pv1 = row(f"pv1{b}")
                for dst, wcol, bias, src in (
                    (magd, w2m_s, b2m_s, hm),
                    (pv0, w2p0_s, b2p0_s, hp),
                    (pv1, w2p1_s, b2p1_s, hp),
                ):
                    for hf in range(2):
                        cs = slice(hf * HD, (hf + 1) * HD)
                        red = pmisc.tile([1, HD], F32, name=f"red{b}{hf}", tag="m")
                        nc.tensor.matmul(red, wcol, src[:, cs])
                        nc.vector.tensor_scalar(dst[:, cs], red, bias, None, AL.add)

                # normalize pv, r_out, A_new
                q0 = row(f"q0{b}")
                nc.scalar.activation(q0, pv0, AF.Square)
                q1 = row(f"q1{b}")
                nc.scalar.activation(q1, pv1, AF.Square)
                nc.vector.tensor_add(q0, q0, q1)
                nc.scalar.activation(q1, q0, AF.Sqrt)
                nc.vector.tensor_scalar(q1, q1, 1e-12, None, AL.max)
                nc.vector.reciprocal(q0, q1)  # q0 = 1/max(norm, 1e-12)
                nc.vector.tensor_mul(pv0, pv0, q0)
                nc.vector.tensor_mul(pv1, pv1, q0)

                nc.vector.scalar_tensor_tensor(
                    out=magd, in0=magd, scalar=msc_s, in1=logm,
                    op0=AL.mult, op1=AL.add,
                )  # magd = log_mag + msc*mag_delta
                r_out = row(f"r{b}")
                nc.scalar.activation(r_out, magd, AF.Exp)
                anr = row(f"anr{b}")
                nc.vector.tensor_mul(anr, r_out, pv0)
                ani = row(f"ani{b}")
                nc.vector.tensor_mul(ani, r_out, pv1)
                return anr, ani

            def replicate(b, ri, src):
                """(1, D) row -> (128, D) sbuf tile via ones-weights matmul."""
                at = arepp.tile(
                    [128, D], F32, name=f"arep{ri}{b}", tag=f"arep{ri}{b}"
                )
                for hf in range(2):
                    cs = slice(hf * HD, (hf + 1) * HD)
                    rs = pmisc.tile([128, HD], F32, name=f"rs{ri}{b}{hf}", tag="m")
                    nc.tensor.matmul(rs, ones128, src[:, cs])
                    nc.vector.tensor_copy(at[:, cs], rs)
                return at

            def phase_c_chunk(b, ri, zsrc, at, i, reused):
                # loads on the SP HWDGE ring, stores on SWDGE: stalled
                # stores must not block later loads (rings are in-order),
                # and the ACT ring must stay free for phase-B activations
                sl = slice(i * TCHUNK, (i + 1) * TCHUNK)
                if reused is not None:
                    zt = reused.bitcast(F32)  # still resident from phase A
                else:
                    zt = zpool.tile([128, KSUB, D], F32, name=f"zo{ri}{b}{i}", tag="z")
                    nc.sync.dma_start(
                        out=zt, in_=zsrc[b][sl].rearrange("(k p) d -> p k d", p=128)
                    )
                for k in range(KSUB):
                    nc.vector.tensor_add(zt[:, k, :], zt[:, k, :], at)
                nc.gpsimd.dma_start(
                    out=out[ri, b][sl].rearrange("(k p) d -> p k d", p=128),
                    in_=zt,
                )

            # ---- schedule: A0 A1 B0 B1 C(reused) C0 C1 ----
            # B-chain latency hides under C-load prefetch; all loads flow
            # through one SP ring gated only by z-pool slot reuse. The last
            # KEEP_CHUNKS chunk-pairs of the last batch stay in SBUF after
            # phase A and are added+stored directly (no re-read from HBM).
            bl = NB - 1
            for _rep in range(reps):
                keep = {}
                acc = [phase_a(b, keep if b == bl else None) for b in range(NB)]
                ats = []
                for b in range(NB):
                    anr, ani = phase_b(b, *acc[b])
                    ats.append((replicate(b, 0, anr), replicate(b, 1, ani)))
                for ri, zsrc in ((0, zr), (1, zi)):
                    for i in range(NCH - KEEP_CHUNKS, NCH):
                        phase_c_chunk(bl, ri, zsrc, ats[bl][ri], i, keep[(ri, i)])
                for b in range(NB):
                    for ri, zsrc in ((0, zr), (1, zi)):
                        for i in range(NCH):
                            if b == bl and i >= NCH - KEEP_CHUNKS:
                                continue
                            phase_c_chunk(b, ri, zsrc, ats[b][ri], i, None)

    nc.compile()
    return nc


_NC = None


def _get_nc():
    global _NC
    if _NC is None:
        _NC = build_kernel()
    return _NC


def make_in_maps(Z_real, Z_imag, mask, W1m, b1m, W2m, b2m, W1p, b1p, W2p, b2p,
                 mag_scale):
    f = np.float32
    consts = {
        "w1m": np.ascontiguousarray(np.asarray(W1m, f).reshape(H, 1)),
        "b1m": np.ascontiguousarray(np.asarray(b1m, f).reshape(H, 1)),
        "w2m": np.ascontiguousarray(np.asarray(W2m, f).reshape(H, 1)),
        "w1p0": np.ascontiguousarray(np.asarray(W1p, f)[:, 0].reshape(H, 1)),
        "w1p1": np.ascontiguousarray(np.asarray(W1p, f)[:, 1].reshape(H, 1)),
        "b1p": np.ascontiguousarray(np.asarray(b1p, f).reshape(H, 1)),
        "w2p0": np.ascontiguousarray(np.asarray(W2p, f)[0].reshape(H, 1)),
        "w2p1": np.ascontiguousarray(np.asarray(W2p, f)[1].reshape(H, 1)),
        "b2m": np.asarray(b2m, f).reshape(1, 1).copy(),
        "b2p0": np.asarray(b2p, f).reshape(-1)[0].reshape(1, 1).copy(),
        "b2p1": np.asarray(b2p, f).reshape(-1)[1].reshape(1, 1).copy(),
        "msc": np.asarray(mag_scale, f).reshape(1, 1).copy(),
    }
    Z_real = np.asarray(Z_real, f)
    Z_imag = np.asarray(Z_imag, f)
    mask = np.asarray(mask, f)
    in_maps = []
    for c in range(NCORES):
        bs = slice(c * NB, (c + 1) * NB)
        in_maps.append(
            {
                "zr": np.ascontiguousarray(Z_real[bs]),
                "zi": np.ascontiguousarray(Z_imag[bs]),
                "mask": np.ascontiguousarray(mask[bs]),
                **consts,
            }
        )
    return in_maps


def kernel(**inputs):
    nc = _get_nc()
    in_maps = make_in_maps(**inputs)
    res = run_bass_kernel_spmd(nc, in_maps, core_ids=list(range(NCORES)))
    out = np.empty((2, B, T, D), np.float32)
    for c in range(NCORES):
        out[:, c * NB : (c + 1) * NB] = res.results[c]["out"]
    return out
